# revision 30
# baseline (speedup 1.0000x reference)
"""Trainium2 Bass kernel for nn_MixtureExpertsMlp (MoE soft routing).

Contract: kernel(**inputs) takes the FULL unsharded inputs
(x [4,4096,768], phi [4,1024,768], w1 [4,768,3072], b1 [4,3072],
w2 [4,3072,768], b2 [4,768]) and returns the FULL output [4,4096,768].

Sharding (expert+slot parallel over 8 NeuronCores): core c owns expert
e = c // 2 and slot half h = c % 2, i.e. SL = 512 of that expert's 1024
routing slots. Every core sees all tokens. Per core and per batch b:

  L^T[s, n]    = sum_d phi[s, d] x[b, n, d]        (slots on partitions)
  E^T          = exp(L^T)          (softmax max-subtraction skipped: the
                                    logits are ~N(0,1), well within fp32)
  ddenom[s]    = sum_n E^T[s, n]                    (via ACT accum_out)
  D[n, s]      = E^T[s, n] / ddenom[s]   (dispatch; per-s 128x128
                                    transpose-matmuls against diag(1/dd))
  slots^T[d,s] = sum_n x[b, n, d] D[n, s]
  h^T[h', s]   = gelu_tanh(sum_d w1[d, h'] slots^T[d, s] + b1[h'])
  y^T[d, s]    = sum_h w2[h, d] h^T[h, s]     (then PE-transposed to y)
  outp[n, :D]  = sum_s E^T[s, n] y[s, :]      (unnormalized combine)
  outp[n, D]   = sum_s E^T[s, n]              (ones column appended to y)

Host-side unshard: the combine softmax normalizer is global over all
E*S slots, so out = (sum_c num_c + sum_c gdl_c * b2[e(c)]) / sum_c gdl_c
where num_c = outp_c[..., :D] and gdl_c = outp_c[..., D]. This also
folds in b2 exactly (per-expert combine mass times b2[e]).

All matmul operands are bf16 (fp32 PSUM accumulation): same PE stream
rate as fp32r but weight loads ride the fast-weight-load path and all
input DMA traffic is halved. w1/w2/phi are loaded to SBUF once and
reused across batches.
"""

import numpy as np
from contextlib import ExitStack

import concourse.bass as bass
import concourse.tile as tile
from concourse import mybir
from concourse.bass import ts
from concourse.masks import make_identity
from concourse.bass_utils import run_bass_kernel_spmd

F32 = mybir.dt.float32
BF16 = mybir.dt.bfloat16
AF = mybir.ActivationFunctionType

N_CORES = 8


# --------------------------------------------------------------------------
# Post-pass: the walrus build in this container enforces the ISA cap of one
# sync-wait per instruction (two for EventSemaphore); Tile's final drain can
# carry more. Hoist excess waits onto fresh same-engine NOPs.
# --------------------------------------------------------------------------
def _split_excess_waits(nc):
    caps = {"InstEventSemaphore": 2}
    n_new = 0
    for f in nc.m.functions:
        for bb in f.blocks:
            i = 0
            insts = bb.instructions
            while i < len(insts):
                ins = insts[i]
                si = ins.sync_info
                cap = caps.get(type(ins).__name__, 1)
                if si is not None and len(si.on_wait) > cap:
                    waits = list(si.on_wait)
                    keep, hoist = waits[-cap:], waits[:-cap]
                    new_nops = []
                    for w in hoist:
                        nop = mybir.InstNoOp(
                            name=nc.get_next_instruction_name(),
                            engine=ins.engine,
                            ins=[],
                            outs=[],
                            sync_info=mybir.SyncInfo(on_wait=[w], on_update=[]),
                        )
                        nc.register_instruction(nop)
                        new_nops.append(nop)
                    ins.sync_info = mybir.SyncInfo(
                        on_wait=keep, on_update=list(si.on_update)
                    )
                    insts[i:i] = new_nops
                    i += len(new_nops)
                    n_new += len(new_nops)
                i += 1
    return n_new


def _emit_moe_kernel(nc, B, N, D, SL, H, act_fn=AF.Gelu_apprx_tanh):
    assert N % 512 == 0 and D % 128 == 0 and SL % 128 == 0 and H % 128 == 0
    Dc, SLc, Hc = D // 128, SL // 128, H // 128
    NT, NV = N // 512, N // 128
    OD = D + 2  # ones column (combine denom) + even-size pad

    xT = nc.dram_tensor("xT", [B, Dc, 128, N], BF16, kind="ExternalInput").ap()
    xN = nc.dram_tensor("xN", [B, N, D], BF16, kind="ExternalInput").ap()
    phiT = nc.dram_tensor("phiT", [Dc, 128, SL], BF16, kind="ExternalInput").ap()
    w1 = nc.dram_tensor("w1", [D, H], BF16, kind="ExternalInput").ap()
    w2 = nc.dram_tensor("w2", [H, D], BF16, kind="ExternalInput").ap()
    b1 = nc.dram_tensor("b1", [Hc, 128], F32, kind="ExternalInput").ap()
    outp = nc.dram_tensor("outp", [B, N, OD], F32, kind="ExternalOutput").ap()

    with tile.TileContext(nc) as tc, ExitStack() as ctx:
        pool = lambda name, bufs, space="SBUF": ctx.enter_context(
            tc.tile_pool(name=name, bufs=bufs, space=space)
        )
        singles = pool("singles", 1)
        eT_pool = pool("eT", 10)
        xT_pool = pool("xT", 2)
        xN_pool = pool("xN", 3)
        D_pool = pool("D", 3)
        slots_pool = pool("slots", 1)
        h_pool = pool("h", 3)
        y_pool = pool("y", 2)
        dd_pool = pool("dd", 2)
        diag_pool = pool("diag", 2)
        out_pool = pool("out", 3)

        # PSUM: 8 banks of 512 f32. Tag "pss": 2 rotating 1-bank tiles for
        # the P1/P3 short-lived accumulators. Tag "acc": flat 6-bank region
        # time-shared by slots^T accumulation (P2), y^T accumulation (P3),
        # and the triple-buffered combine outputs (P4).
        ps_small = pool("ps_small", 2, "PSUM")
        ps_acc = pool("ps_acc", 1, "PSUM")
        ACC = Dc * 512
        assert ACC * 4 <= 6 * 2048

        phiT_s = singles.tile([128, Dc, SL], BF16)
        nc.sync.dma_start(phiT_s[:], phiT.rearrange("k p m -> p k m"))
        b1_s = singles.tile([128, Hc], F32)
        nc.sync.dma_start(b1_s[:], b1.rearrange("o p -> p o"))
        # w1/w2 (9.4 MB) are DMAed after batch 0's phase-1 instructions are
        # issued, so the first x tiles aren't queued behind them; the load
        # still completes well before phase 3 needs the weights.
        w1_s = singles.tile([128, Dc, H], BF16)
        w2_s = singles.tile([128, Hc, D], BF16)
        ident = singles.tile([128, 128], F32)
        make_identity(nc, ident[:])
        zbias = singles.tile([128, 1], F32)
        nc.vector.memset(zbias[:], 0.0)

        for b in range(B):
            # ---- phase 1: logits + exp -> E^T tiles, with exp-sums ----
            eT_t = []
            ddp = dd_pool.tile([128, SLc, NT], F32)
            for t in range(NT):
                et = eT_pool.tile([128, SLc, 512], BF16, name="et")
                eT_t.append(et)
                xt = xT_pool.tile([128, Dc, 512], BF16)
                nc.sync.dma_start(
                    xt[:], xT[b, :, :, ts(t, 512)].rearrange("k p n -> p k n")
                )
                for s in range(SLc):
                    ps = ps_small.tile([128, 512], F32, tag="pss", name="psL")
                    for d in range(Dc):
                        nc.tensor.matmul(
                            ps[:],
                            phiT_s[:, d, ts(s, 128)],
                            xt[:, d, :],
                            start=(d == 0),
                            stop=(d == Dc - 1),
                        )
                    nc.scalar.activation(
                        et[:, s, :],
                        ps[:],
                        AF.Exp,
                        bias=zbias[:],
                        accum_out=ddp[:, s, t : t + 1],
                    )

            if b == 0:
                w1_r = w1.rearrange("(k p) m -> p k m", p=128)
                w2_r = w2.rearrange("(k p) m -> p k m", p=128)
                for d in range(Dc):
                    nc.sync.dma_start(w1_s[:, d, :], w1_r[:, d, :])
                for h4 in range(0, Hc, 4):
                    nc.sync.dma_start(
                        w2_s[:, h4 : h4 + 4, :], w2_r[:, h4 : h4 + 4, :]
                    )

            def eT_blk(s, v):
                return eT_t[v // 4][:, s, ts(v % 4, 128)]

            # ---- dispatch denominators -> per-s scaled identity ----
            rdd = dd_pool.tile([128, SLc], F32)
            nc.vector.reduce_sum(rdd[:], ddp[:], axis=mybir.AxisListType.X)
            nc.vector.reciprocal(rdd[:], rdd[:])
            diag = diag_pool.tile([128, SLc, 128], BF16)
            for s in range(SLc):
                nc.vector.tensor_scalar_mul(
                    diag[:, s, :], ident[:], rdd[:, s : s + 1]
                )
            # ---- phase 2: dispatch transpose+normalize, slots^T matmul ----
            # The transpose-matmuls against diag(1/dd) fuse the softmax
            # normalization into the E^T block transposes. Software-pipelined
            # one v ahead so the PE runs v+1 transposes while the DVE drains
            # psDt(v) into Dt(v).
            accS = ps_acc.tile([128, ACC], F32, tag="acc", name="accS")

            def p2_transposes(v):
                psDt = ps_small.tile([128, 512], F32, tag="pss", name="psD")
                for s in range(SLc):
                    nc.tensor.matmul(
                        psDt[:, ts(s, 128)],
                        eT_blk(s, v),
                        diag[:, s, :],
                        start=True,
                        stop=True,
                    )
                Dt = D_pool.tile([128, SL], BF16)
                nc.vector.tensor_copy(Dt[:], psDt[:])
                xn = xN_pool.tile([128, D], BF16)
                nc.sync.dma_start(xn[:], xN[b, ts(v, 128), :])
                return Dt, xn

            def p2_slots(v, Dt, xn):
                for d in range(Dc):
                    nc.tensor.matmul(
                        accS[:, d * 512 : d * 512 + SL],
                        xn[:, ts(d, 128)],
                        Dt[:],
                        start=(v == 0),
                        stop=(v == NV - 1),
                    )

            pend = [p2_transposes(0), p2_transposes(1)]
            for v in range(NV):
                if v + 2 < NV:
                    pend.append(p2_transposes(v + 2))
                p2_slots(v, *pend.pop(0))
            slotsT = slots_pool.tile([128, Dc, SL], BF16)
            for d in range(Dc):
                nc.vector.tensor_copy(
                    slotsT[:, d, :], accS[:, d * 512 : d * 512 + SL]
                )
            # ---- phase 3: expert MLP, y^T accumulation ----
            # y matmuls for h trail the h matmuls for h+1 by one step so the
            # PE never waits on gelu.
            accY = ps_acc.tile([128, ACC], F32, tag="acc", name="accY")

            def p3_h(h):
                psh = ps_small.tile([128, 512], F32, tag="pss", name="psH")
                for d in range(Dc):
                    nc.tensor.matmul(
                        psh[:, :SL],
                        w1_s[:, d, ts(h, 128)],
                        slotsT[:, d, :],
                        start=(d == 0),
                        stop=(d == Dc - 1),
                    )
                ht = h_pool.tile([128, SL], BF16)
                nc.scalar.activation(
                    ht[:], psh[:, :SL], act_fn, bias=b1_s[:, h : h + 1]
                )
                return ht

            def p3_y(h, ht):
                for d in range(Dc):
                    nc.tensor.matmul(
                        accY[:, d * 512 : d * 512 + SL],
                        w2_s[:, h, ts(d, 128)],
                        ht[:],
                        start=(h == 0),
                        stop=(h == Hc - 1),
                    )

            pend_h = p3_h(0)
            for h in range(Hc):
                nxt_h = p3_h(h + 1) if h + 1 < Hc else None
                p3_y(h, pend_h)
                pend_h = nxt_h
            # y^T -> y via one XBAR transpose: yTs free layout (s, d, i)
            # so yTT blocks come out (s, d)-ordered and slot block s reads
            # its 768 y columns contiguously. The 256-wide tail + the ones
            # column are staged into y_augB for the combine pB group.
            yTs = slots_pool.tile([128, SLc, Dc, 128], BF16, tag="yTs", name="yTs")
            for d in range(Dc):
                nc.vector.tensor_copy(
                    yTs[:, :, d, :],
                    accY[:, d * 512 : d * 512 + SL].rearrange(
                        "p (s k) -> p s k", s=SLc
                    ),
                )
            yTT = y_pool.tile([128, SLc * Dc, 128], BF16, tag="yTT", name="yTT")
            nc.sync.dma_start_transpose(yTT[:], yTs[:])
            y_augB = y_pool.tile([128, SLc, OD - 512], BF16, tag="yB", name="yB")
            nc.vector.memset(y_augB[:, :, 256 : 257], 1.0)
            nc.vector.memset(y_augB[:, :, 257 : 258], 0.0)
            nc.vector.tensor_copy(
                y_augB[:, :, 0:256],
                yTT[:, :, :].rearrange("p (s d) k -> p s (d k)", s=SLc)[
                    :, :, 512:D
                ],
            )
            # ---- phase 4: combine partials + local denominator ----
            # Triple-buffered by column ranges of the 6-bank acc region; the
            # drain copies go to separate tiles on separate engines.
            psC = ps_acc.tile([128, ACC], F32, tag="acc", name="psC")

            def p4_mms(v):
                base = (v % 3) * 1024
                pA = psC[:, base : base + 512]
                pB = psC[:, base + 512 : base + OD]
                for s in range(SLc):
                    nc.tensor.matmul(
                        pA,
                        eT_blk(s, v),
                        yTT[:, ts(s, Dc), :].rearrange("p d k -> p (d k)")[
                            :, 0:512
                        ],
                        start=(s == 0),
                        stop=(s == SLc - 1),
                    )
                    nc.tensor.matmul(
                        pB,
                        eT_blk(s, v),
                        y_augB[:, s, :],
                        start=(s == 0),
                        stop=(s == SLc - 1),
                    )
                return pA, pB

            def p4_drain(v, pA, pB):
                ot = out_pool.tile([128, OD], F32, tag="ot", name="ot")
                base = (v % 3) * 1024
                if v % 3 == 0:
                    nc.scalar.copy(ot[:], psC[:, base : base + OD])
                elif v % 3 == 1:
                    nc.vector.tensor_copy(ot[:], psC[:, base : base + OD])
                else:
                    hf = OD // 2
                    nc.scalar.copy(ot[:, :hf], psC[:, base : base + hf])
                    nc.vector.tensor_copy(
                        ot[:, hf:], psC[:, base + hf : base + OD]
                    )
                nc.sync.dma_start(outp[b, ts(v, 128), :], ot[:])

            # Groups of three v-blocks fill the three 2-bank regions, then
            # drain on three different engines in parallel; only the slowest
            # drain gates the next group.
            for v0 in range(0, NV, 3):
                grp = [(v, p4_mms(v)) for v in range(v0, min(v0 + 3, NV))]
                for v, ps in grp:
                    p4_drain(v, *ps)

    return nc


def _to_bf16(a):
    import ml_dtypes

    return np.asarray(a, dtype=np.float32).astype(ml_dtypes.bfloat16)


def _make_core_inputs(x, phi, w1, b1, w2, n_cores=N_CORES):
    B, N, Dd = x.shape
    E, S, _ = phi.shape
    H = w1.shape[2]
    halves = n_cores // E
    SL = S // halves
    Dc, Hc = Dd // 128, H // 128
    xT_full = _to_bf16(
        np.ascontiguousarray(x.transpose(0, 2, 1)).reshape(B, Dc, 128, N)
    )
    x_c = _to_bf16(x)
    w1_b = [_to_bf16(w1[e]) for e in range(E)]
    w2_b = [_to_bf16(w2[e]) for e in range(E)]
    b1_b = [np.ascontiguousarray(b1[e]).reshape(Hc, 128) for e in range(E)]
    in_maps = []
    for c in range(n_cores):
        e, hh = c // halves, c % halves
        phi_loc = phi[e, hh * SL : (hh + 1) * SL, :]
        phiT = _to_bf16(np.ascontiguousarray(phi_loc.T).reshape(Dc, 128, SL))
        in_maps.append(
            {
                "xT": xT_full,
                "xN": x_c,
                "phiT": phiT,
                "w1": w1_b[e],
                "w2": w2_b[e],
                "b1": b1_b[e],
            }
        )
    return in_maps


def _combine_core_outputs(outs, b2, n_cores=N_CORES):
    E, D = b2.shape
    halves = n_cores // E
    num = np.zeros(outs[0]["outp"][..., :D].shape, dtype=np.float64)
    den = np.zeros(outs[0]["outp"][..., D].shape, dtype=np.float64)
    for c, r in enumerate(outs):
        e = c // halves
        gdl = r["outp"][..., D].astype(np.float64)
        num += r["outp"][..., :D]
        num += gdl[..., None] * b2[e].astype(np.float64)[None, None, :]
        den += gdl
    return (num / den[..., None]).astype(np.float32)


def build_nc(x, phi, w1, b1, w2, b2):
    B, N, D = x.shape
    E, S, _ = phi.shape
    H = w1.shape[2]
    SL = S // (N_CORES // E)

    nc = bass.Bass(
        "TRN2", target_bir_lowering=False, debug=False, num_devices=N_CORES
    )
    _emit_moe_kernel(nc, B, N, D, SL, H)
    _split_excess_waits(nc)
    return nc


def make_in_maps(x, phi, w1, b1, w2, b2):
    return _make_core_inputs(
        np.asarray(x, dtype=np.float32),
        np.asarray(phi, dtype=np.float32),
        np.asarray(w1, dtype=np.float32),
        np.asarray(b1, dtype=np.float32),
        np.asarray(w2, dtype=np.float32),
    )


def combine_outputs(results, x, phi, w1, b1, w2, b2):
    return _combine_core_outputs(results, np.asarray(b2, dtype=np.float32))


def kernel(x, phi, w1, b1, w2, b2):
    x = np.asarray(x, dtype=np.float32)
    phi = np.asarray(phi, dtype=np.float32)
    w1 = np.asarray(w1, dtype=np.float32)
    b1 = np.asarray(b1, dtype=np.float32)
    w2 = np.asarray(w2, dtype=np.float32)
    b2 = np.asarray(b2, dtype=np.float32)

    nc = build_nc(x, phi, w1, b1, w2, b2)
    in_maps = make_in_maps(x, phi, w1, b1, w2, b2)
    res = run_bass_kernel_spmd(nc, in_maps, core_ids=list(range(N_CORES)))
    return combine_outputs(res.results, x, phi, w1, b1, w2, b2)


# revision 31
# speedup vs baseline: 1.0013x; 1.0013x over previous
"""Trainium2 Bass kernel for nn_MixtureExpertsMlp (MoE soft routing).

Contract: kernel(**inputs) takes the FULL unsharded inputs
(x [4,4096,768], phi [4,1024,768], w1 [4,768,3072], b1 [4,3072],
w2 [4,3072,768], b2 [4,768]) and returns the FULL output [4,4096,768].

Sharding (expert+slot parallel over 8 NeuronCores): core c owns expert
e = c // 2 and slot half h = c % 2, i.e. SL = 512 of that expert's 1024
routing slots. Every core sees all tokens. Per core and per batch b:

  L^T[s, n]    = sum_d phi[s, d] x[b, n, d]        (slots on partitions)
  E^T          = exp(L^T)          (softmax max-subtraction skipped: the
                                    logits are ~N(0,1), well within fp32)
  ddenom[s]    = sum_n E^T[s, n]                    (via ACT accum_out)
  D[n, s]      = E^T[s, n] / ddenom[s]   (dispatch; per-s 128x128
                                    transpose-matmuls against diag(1/dd))
  slots^T[d,s] = sum_n x[b, n, d] D[n, s]
  h^T[h', s]   = gelu_tanh(sum_d w1[d, h'] slots^T[d, s] + b1[h'])
  y^T[d, s]    = sum_h w2[h, d] h^T[h, s]     (then PE-transposed to y)
  outp[n, :D]  = sum_s E^T[s, n] y[s, :]      (unnormalized combine)
  outp[n, D]   = sum_s E^T[s, n]              (ones column appended to y)

Host-side unshard: the combine softmax normalizer is global over all
E*S slots, so out = (sum_c num_c + sum_c gdl_c * b2[e(c)]) / sum_c gdl_c
where num_c = outp_c[..., :D] and gdl_c = outp_c[..., D]. This also
folds in b2 exactly (per-expert combine mass times b2[e]).

All matmul operands are bf16 (fp32 PSUM accumulation): same PE stream
rate as fp32r but weight loads ride the fast-weight-load path and all
input DMA traffic is halved. w1/w2/phi are loaded to SBUF once and
reused across batches.
"""

import numpy as np
from contextlib import ExitStack

import concourse.bass as bass
import concourse.tile as tile
from concourse import mybir
from concourse.bass import ts
from concourse.masks import make_identity
from concourse.bass_utils import run_bass_kernel_spmd

F32 = mybir.dt.float32
BF16 = mybir.dt.bfloat16
AF = mybir.ActivationFunctionType

N_CORES = 8


# --------------------------------------------------------------------------
# Post-pass: the walrus build in this container enforces the ISA cap of one
# sync-wait per instruction (two for EventSemaphore); Tile's final drain can
# carry more. Hoist excess waits onto fresh same-engine NOPs.
# --------------------------------------------------------------------------
def _split_excess_waits(nc):
    caps = {"InstEventSemaphore": 2}
    n_new = 0
    for f in nc.m.functions:
        for bb in f.blocks:
            i = 0
            insts = bb.instructions
            while i < len(insts):
                ins = insts[i]
                si = ins.sync_info
                cap = caps.get(type(ins).__name__, 1)
                if si is not None and len(si.on_wait) > cap:
                    waits = list(si.on_wait)
                    keep, hoist = waits[-cap:], waits[:-cap]
                    new_nops = []
                    for w in hoist:
                        nop = mybir.InstNoOp(
                            name=nc.get_next_instruction_name(),
                            engine=ins.engine,
                            ins=[],
                            outs=[],
                            sync_info=mybir.SyncInfo(on_wait=[w], on_update=[]),
                        )
                        nc.register_instruction(nop)
                        new_nops.append(nop)
                    ins.sync_info = mybir.SyncInfo(
                        on_wait=keep, on_update=list(si.on_update)
                    )
                    insts[i:i] = new_nops
                    i += len(new_nops)
                    n_new += len(new_nops)
                i += 1
    return n_new


def _emit_moe_kernel(nc, B, N, D, SL, H, act_fn=AF.Gelu_apprx_tanh):
    assert N % 512 == 0 and D % 128 == 0 and SL % 128 == 0 and H % 128 == 0
    Dc, SLc, Hc = D // 128, SL // 128, H // 128
    NT, NV = N // 512, N // 128
    OD = D + 2  # ones column (combine denom) + even-size pad

    xT = nc.dram_tensor("xT", [B, Dc, 128, N], BF16, kind="ExternalInput").ap()
    xN = nc.dram_tensor("xN", [B, N, D], BF16, kind="ExternalInput").ap()
    phiT = nc.dram_tensor("phiT", [Dc, 128, SL], BF16, kind="ExternalInput").ap()
    w1 = nc.dram_tensor("w1", [D, H], BF16, kind="ExternalInput").ap()
    w2 = nc.dram_tensor("w2", [H, D], BF16, kind="ExternalInput").ap()
    b1 = nc.dram_tensor("b1", [Hc, 128], F32, kind="ExternalInput").ap()
    outp = nc.dram_tensor("outp", [B, N, OD], F32, kind="ExternalOutput").ap()

    with tile.TileContext(nc) as tc, ExitStack() as ctx:
        pool = lambda name, bufs, space="SBUF": ctx.enter_context(
            tc.tile_pool(name=name, bufs=bufs, space=space)
        )
        singles = pool("singles", 1)
        eT_pool = pool("eT", 10)
        xT_pool = pool("xT", 2)
        xN_pool = pool("xN", 3)
        D_pool = pool("D", 3)
        slots_pool = pool("slots", 1)
        h_pool = pool("h", 3)
        y_pool = pool("y", 2)
        dd_pool = pool("dd", 2)
        diag_pool = pool("diag", 2)
        out_pool = pool("out", 3)

        # PSUM: 8 banks of 512 f32. Tag "pss": 2 rotating 1-bank tiles for
        # the P1/P3 short-lived accumulators. Tag "acc": flat 6-bank region
        # time-shared by slots^T accumulation (P2), y^T accumulation (P3),
        # and the triple-buffered combine outputs (P4).
        ps_small = pool("ps_small", 2, "PSUM")
        ps_acc = pool("ps_acc", 1, "PSUM")
        ACC = Dc * 512
        assert ACC * 4 <= 6 * 2048

        phiT_s = singles.tile([128, Dc, SL], BF16)
        nc.sync.dma_start(phiT_s[:], phiT.rearrange("k p m -> p k m"))
        b1_s = singles.tile([128, Hc], F32)
        nc.sync.dma_start(b1_s[:], b1.rearrange("o p -> p o"))
        # w1/w2 (9.4 MB) are DMAed after batch 0's phase-1 instructions are
        # issued, so the first x tiles aren't queued behind them; the load
        # still completes well before phase 3 needs the weights.
        w1_s = singles.tile([128, Dc, H], BF16)
        w2_s = singles.tile([128, Hc, D], BF16)
        ident = singles.tile([128, 128], F32)
        make_identity(nc, ident[:])
        zbias = singles.tile([128, 1], F32)
        nc.vector.memset(zbias[:], 0.0)

        for b in range(B):
            # ---- phase 1: logits + exp -> E^T tiles, with exp-sums ----
            eT_t = []
            ddp = dd_pool.tile([128, SLc, NT], F32)
            for t in range(NT):
                et = eT_pool.tile([128, SLc, 512], BF16, name="et")
                eT_t.append(et)
                xt = xT_pool.tile([128, Dc, 512], BF16)
                nc.sync.dma_start(
                    xt[:], xT[b, :, :, ts(t, 512)].rearrange("k p n -> p k n")
                )
                for s in range(SLc):
                    ps = ps_small.tile([128, 512], F32, tag="pss", name="psL")
                    for d in range(Dc):
                        nc.tensor.matmul(
                            ps[:],
                            phiT_s[:, d, ts(s, 128)],
                            xt[:, d, :],
                            start=(d == 0),
                            stop=(d == Dc - 1),
                        )
                    nc.scalar.activation(
                        et[:, s, :],
                        ps[:],
                        AF.Exp,
                        bias=zbias[:],
                        accum_out=ddp[:, s, t : t + 1],
                    )

            if b == 0:
                w1_r = w1.rearrange("(k p) m -> p k m", p=128)
                w2_r = w2.rearrange("(k p) m -> p k m", p=128)
                for d in range(Dc):
                    nc.sync.dma_start(w1_s[:, d, :], w1_r[:, d, :])
                for h4 in range(0, Hc, 4):
                    nc.sync.dma_start(
                        w2_s[:, h4 : h4 + 4, :], w2_r[:, h4 : h4 + 4, :]
                    )

            def eT_blk(s, v):
                return eT_t[v // 4][:, s, ts(v % 4, 128)]

            # ---- dispatch denominators -> per-s scaled identity ----
            rdd = dd_pool.tile([128, SLc], F32)
            nc.vector.reduce_sum(rdd[:], ddp[:], axis=mybir.AxisListType.X)
            nc.vector.reciprocal(rdd[:], rdd[:])
            diag = diag_pool.tile([128, SLc, 128], BF16)
            for s in range(SLc):
                nc.vector.tensor_scalar_mul(
                    diag[:, s, :], ident[:], rdd[:, s : s + 1]
                )
            # ---- phase 2: dispatch transpose+normalize, slots^T matmul ----
            # The transpose-matmuls against diag(1/dd) fuse the softmax
            # normalization into the E^T block transposes. Software-pipelined
            # one v ahead so the PE runs v+1 transposes while the DVE drains
            # psDt(v) into Dt(v).
            accS = ps_acc.tile([128, ACC], F32, tag="acc", name="accS")

            def p2_transposes(v):
                psDt = ps_small.tile([128, 512], F32, tag="pss", name="psD")
                for s in range(SLc):
                    nc.tensor.matmul(
                        psDt[:, ts(s, 128)],
                        eT_blk(s, v),
                        diag[:, s, :],
                        start=True,
                        stop=True,
                    )
                Dt = D_pool.tile([128, SL], BF16)
                nc.vector.tensor_copy(Dt[:], psDt[:])
                xn = xN_pool.tile([128, D], BF16)
                nc.sync.dma_start(xn[:], xN[b, ts(v, 128), :])
                return Dt, xn

            def p2_slots(v, Dt, xn):
                for d in range(Dc):
                    nc.tensor.matmul(
                        accS[:, d * 512 : d * 512 + SL],
                        xn[:, ts(d, 128)],
                        Dt[:],
                        start=(v == 0),
                        stop=(v == NV - 1),
                    )

            pend = p2_transposes(0)
            for v in range(NV):
                nxt = p2_transposes(v + 1) if v + 1 < NV else None
                p2_slots(v, *pend)
                pend = nxt
            slotsT = slots_pool.tile([128, Dc, SL], BF16)
            for d in range(Dc):
                nc.vector.tensor_copy(
                    slotsT[:, d, :], accS[:, d * 512 : d * 512 + SL]
                )
            # ---- phase 3: expert MLP, y^T accumulation ----
            # y matmuls for h trail the h matmuls for h+1 by one step so the
            # PE never waits on gelu.
            accY = ps_acc.tile([128, ACC], F32, tag="acc", name="accY")

            def p3_h(h):
                psh = ps_small.tile([128, 512], F32, tag="pss", name="psH")
                for d in range(Dc):
                    nc.tensor.matmul(
                        psh[:, :SL],
                        w1_s[:, d, ts(h, 128)],
                        slotsT[:, d, :],
                        start=(d == 0),
                        stop=(d == Dc - 1),
                    )
                ht = h_pool.tile([128, SL], BF16)
                nc.scalar.activation(
                    ht[:], psh[:, :SL], act_fn, bias=b1_s[:, h : h + 1]
                )
                return ht

            def p3_y(h, ht):
                for d in range(Dc):
                    nc.tensor.matmul(
                        accY[:, d * 512 : d * 512 + SL],
                        w2_s[:, h, ts(d, 128)],
                        ht[:],
                        start=(h == 0),
                        stop=(h == Hc - 1),
                    )

            pend_h = p3_h(0)
            for h in range(Hc):
                nxt_h = p3_h(h + 1) if h + 1 < Hc else None
                p3_y(h, pend_h)
                pend_h = nxt_h
            # y^T -> y via one XBAR transpose: yTs free layout (s, d, i)
            # so yTT blocks come out (s, d)-ordered and slot block s reads
            # its 768 y columns contiguously. The 256-wide tail + the ones
            # column are staged into y_augB for the combine pB group.
            yTs = slots_pool.tile([128, SLc, Dc, 128], BF16, tag="yTs", name="yTs")
            for d in range(Dc):
                nc.vector.tensor_copy(
                    yTs[:, :, d, :],
                    accY[:, d * 512 : d * 512 + SL].rearrange(
                        "p (s k) -> p s k", s=SLc
                    ),
                )
            yTT = y_pool.tile([128, SLc * Dc, 128], BF16, tag="yTT", name="yTT")
            nc.sync.dma_start_transpose(yTT[:], yTs[:])
            y_augB = y_pool.tile([128, SLc, OD - 512], BF16, tag="yB", name="yB")
            nc.vector.memset(y_augB[:, :, 256 : 257], 1.0)
            nc.vector.memset(y_augB[:, :, 257 : 258], 0.0)
            nc.vector.tensor_copy(
                y_augB[:, :, 0:256],
                yTT[:, :, :].rearrange("p (s d) k -> p s (d k)", s=SLc)[
                    :, :, 512:D
                ],
            )
            # ---- phase 4: combine partials + local denominator ----
            # Triple-buffered by column ranges of the 6-bank acc region; the
            # drain copies go to separate tiles on separate engines.
            psC = ps_acc.tile([128, ACC], F32, tag="acc", name="psC")

            def p4_mms(v):
                base = (v % 3) * 1024
                pA = psC[:, base : base + 512]
                pB = psC[:, base + 512 : base + OD]
                for s in range(SLc):
                    nc.tensor.matmul(
                        pA,
                        eT_blk(s, v),
                        yTT[:, ts(s, Dc), :].rearrange("p d k -> p (d k)")[
                            :, 0:512
                        ],
                        start=(s == 0),
                        stop=(s == SLc - 1),
                    )
                    nc.tensor.matmul(
                        pB,
                        eT_blk(s, v),
                        y_augB[:, s, :],
                        start=(s == 0),
                        stop=(s == SLc - 1),
                    )
                return pA, pB

            def p4_drain(v, pA, pB):
                ot = out_pool.tile([128, OD], F32, tag="ot", name="ot")
                base = (v % 3) * 1024
                if v % 3 == 0:
                    nc.scalar.copy(ot[:], psC[:, base : base + OD])
                elif v % 3 == 1:
                    nc.vector.tensor_copy(ot[:], psC[:, base : base + OD])
                else:
                    hf = OD // 2
                    nc.scalar.copy(ot[:, :hf], psC[:, base : base + hf])
                    nc.vector.tensor_copy(
                        ot[:, hf:], psC[:, base + hf : base + OD]
                    )
                nc.sync.dma_start(outp[b, ts(v, 128), :], ot[:])

            # Groups of three v-blocks fill the three 2-bank regions, then
            # drain on three different engines in parallel; only the slowest
            # drain gates the next group.
            for v0 in range(0, NV, 3):
                grp = [(v, p4_mms(v)) for v in range(v0, min(v0 + 3, NV))]
                for v, ps in grp:
                    p4_drain(v, *ps)

    return nc


def _to_bf16(a):
    import ml_dtypes

    return np.asarray(a, dtype=np.float32).astype(ml_dtypes.bfloat16)


def _make_core_inputs(x, phi, w1, b1, w2, n_cores=N_CORES):
    B, N, Dd = x.shape
    E, S, _ = phi.shape
    H = w1.shape[2]
    halves = n_cores // E
    SL = S // halves
    Dc, Hc = Dd // 128, H // 128
    xT_full = _to_bf16(
        np.ascontiguousarray(x.transpose(0, 2, 1)).reshape(B, Dc, 128, N)
    )
    x_c = _to_bf16(x)
    w1_b = [_to_bf16(w1[e]) for e in range(E)]
    w2_b = [_to_bf16(w2[e]) for e in range(E)]
    b1_b = [np.ascontiguousarray(b1[e]).reshape(Hc, 128) for e in range(E)]
    in_maps = []
    for c in range(n_cores):
        e, hh = c // halves, c % halves
        phi_loc = phi[e, hh * SL : (hh + 1) * SL, :]
        phiT = _to_bf16(np.ascontiguousarray(phi_loc.T).reshape(Dc, 128, SL))
        in_maps.append(
            {
                "xT": xT_full,
                "xN": x_c,
                "phiT": phiT,
                "w1": w1_b[e],
                "w2": w2_b[e],
                "b1": b1_b[e],
            }
        )
    return in_maps


def _combine_core_outputs(outs, b2, n_cores=N_CORES):
    E, D = b2.shape
    halves = n_cores // E
    num = np.zeros(outs[0]["outp"][..., :D].shape, dtype=np.float64)
    den = np.zeros(outs[0]["outp"][..., D].shape, dtype=np.float64)
    for c, r in enumerate(outs):
        e = c // halves
        gdl = r["outp"][..., D].astype(np.float64)
        num += r["outp"][..., :D]
        num += gdl[..., None] * b2[e].astype(np.float64)[None, None, :]
        den += gdl
    return (num / den[..., None]).astype(np.float32)


def build_nc(x, phi, w1, b1, w2, b2):
    B, N, D = x.shape
    E, S, _ = phi.shape
    H = w1.shape[2]
    SL = S // (N_CORES // E)

    nc = bass.Bass(
        "TRN2", target_bir_lowering=False, debug=False, num_devices=N_CORES
    )
    _emit_moe_kernel(nc, B, N, D, SL, H)
    _split_excess_waits(nc)
    return nc


def make_in_maps(x, phi, w1, b1, w2, b2):
    return _make_core_inputs(
        np.asarray(x, dtype=np.float32),
        np.asarray(phi, dtype=np.float32),
        np.asarray(w1, dtype=np.float32),
        np.asarray(b1, dtype=np.float32),
        np.asarray(w2, dtype=np.float32),
    )


def combine_outputs(results, x, phi, w1, b1, w2, b2):
    return _combine_core_outputs(results, np.asarray(b2, dtype=np.float32))


def kernel(x, phi, w1, b1, w2, b2):
    x = np.asarray(x, dtype=np.float32)
    phi = np.asarray(phi, dtype=np.float32)
    w1 = np.asarray(w1, dtype=np.float32)
    b1 = np.asarray(b1, dtype=np.float32)
    w2 = np.asarray(w2, dtype=np.float32)
    b2 = np.asarray(b2, dtype=np.float32)

    nc = build_nc(x, phi, w1, b1, w2, b2)
    in_maps = make_in_maps(x, phi, w1, b1, w2, b2)
    res = run_bass_kernel_spmd(nc, in_maps, core_ids=list(range(N_CORES)))
    return combine_outputs(res.results, x, phi, w1, b1, w2, b2)


# revision 32
# speedup vs baseline: 1.0138x; 1.0125x over previous
"""Trainium2 Bass kernel for nn_MixtureExpertsMlp (MoE soft routing).

Contract: kernel(**inputs) takes the FULL unsharded inputs
(x [4,4096,768], phi [4,1024,768], w1 [4,768,3072], b1 [4,3072],
w2 [4,3072,768], b2 [4,768]) and returns the FULL output [4,4096,768].

Sharding (expert+slot parallel over 8 NeuronCores): core c owns expert
e = c // 2 and slot half h = c % 2, i.e. SL = 512 of that expert's 1024
routing slots. Every core sees all tokens. Per core and per batch b:

  L^T[s, n]    = sum_d phi[s, d] x[b, n, d]        (slots on partitions)
  E^T          = exp(L^T)          (softmax max-subtraction skipped: the
                                    logits are ~N(0,1), well within fp32)
  ddenom[s]    = sum_n E^T[s, n]                    (via ACT accum_out)
  D[n, s]      = E^T[s, n] / ddenom[s]   (dispatch; per-s 128x128
                                    transpose-matmuls against diag(1/dd))
  slots^T[d,s] = sum_n x[b, n, d] D[n, s]
  h^T[h', s]   = gelu_tanh(sum_d w1[d, h'] slots^T[d, s] + b1[h'])
  y^T[d, s]    = sum_h w2[h, d] h^T[h, s]     (then PE-transposed to y)
  outp[n, :D]  = sum_s E^T[s, n] y[s, :]      (unnormalized combine)
  outp[n, D]   = sum_s E^T[s, n]              (ones column appended to y)

Host-side unshard: the combine softmax normalizer is global over all
E*S slots, so out = (sum_c num_c + sum_c gdl_c * b2[e(c)]) / sum_c gdl_c
where num_c = outp_c[..., :D] and gdl_c = outp_c[..., D]. This also
folds in b2 exactly (per-expert combine mass times b2[e]).

All matmul operands are bf16 (fp32 PSUM accumulation): same PE stream
rate as fp32r but weight loads ride the fast-weight-load path and all
input DMA traffic is halved. w1/w2/phi are loaded to SBUF once and
reused across batches.
"""

import numpy as np
from contextlib import ExitStack

import concourse.bass as bass
import concourse.tile as tile
from concourse import mybir
from concourse.bass import ts
from concourse.masks import make_identity
from concourse.bass_utils import run_bass_kernel_spmd

F32 = mybir.dt.float32
BF16 = mybir.dt.bfloat16
AF = mybir.ActivationFunctionType

N_CORES = 8


# --------------------------------------------------------------------------
# Post-pass: the walrus build in this container enforces the ISA cap of one
# sync-wait per instruction (two for EventSemaphore); Tile's final drain can
# carry more. Hoist excess waits onto fresh same-engine NOPs.
# --------------------------------------------------------------------------
def _split_excess_waits(nc):
    caps = {"InstEventSemaphore": 2}
    n_new = 0
    for f in nc.m.functions:
        for bb in f.blocks:
            i = 0
            insts = bb.instructions
            while i < len(insts):
                ins = insts[i]
                si = ins.sync_info
                cap = caps.get(type(ins).__name__, 1)
                if si is not None and len(si.on_wait) > cap:
                    waits = list(si.on_wait)
                    keep, hoist = waits[-cap:], waits[:-cap]
                    new_nops = []
                    for w in hoist:
                        nop = mybir.InstNoOp(
                            name=nc.get_next_instruction_name(),
                            engine=ins.engine,
                            ins=[],
                            outs=[],
                            sync_info=mybir.SyncInfo(on_wait=[w], on_update=[]),
                        )
                        nc.register_instruction(nop)
                        new_nops.append(nop)
                    ins.sync_info = mybir.SyncInfo(
                        on_wait=keep, on_update=list(si.on_update)
                    )
                    insts[i:i] = new_nops
                    i += len(new_nops)
                    n_new += len(new_nops)
                i += 1
    return n_new


def _emit_moe_kernel(nc, B, N, D, SL, H, act_fn=AF.Gelu_apprx_tanh):
    assert N % 512 == 0 and D % 128 == 0 and SL % 128 == 0 and H % 128 == 0
    Dc, SLc, Hc = D // 128, SL // 128, H // 128
    NT, NV = N // 512, N // 128
    OD = D + 2  # ones column (combine denom) + even-size pad

    xT = nc.dram_tensor("xT", [B, Dc, 128, N], BF16, kind="ExternalInput").ap()
    xN = nc.dram_tensor("xN", [B, N, D], BF16, kind="ExternalInput").ap()
    phiT = nc.dram_tensor("phiT", [Dc, 128, SL], BF16, kind="ExternalInput").ap()
    w1 = nc.dram_tensor("w1", [D, H], BF16, kind="ExternalInput").ap()
    w2 = nc.dram_tensor("w2", [H, D], BF16, kind="ExternalInput").ap()
    b1 = nc.dram_tensor("b1", [Hc, 128], F32, kind="ExternalInput").ap()
    outp = nc.dram_tensor("outp", [B, N, OD], F32, kind="ExternalOutput").ap()

    with tile.TileContext(nc) as tc, ExitStack() as ctx:
        pool = lambda name, bufs, space="SBUF": ctx.enter_context(
            tc.tile_pool(name=name, bufs=bufs, space=space)
        )
        singles = pool("singles", 1)
        eT_pool = pool("eT", 10)
        xT_pool = pool("xT", 2)
        xN_pool = pool("xN", 3)
        D_pool = pool("D", 3)
        slots_pool = pool("slots", 1)
        h_pool = pool("h", 3)
        y_pool = pool("y", 2)
        dd_pool = pool("dd", 2)
        diag_pool = pool("diag", 2)
        out_pool = pool("out", 3)

        # PSUM: 8 banks of 512 f32. Tag "pss": 2 rotating 1-bank tiles for
        # the P1/P3 short-lived accumulators. Tag "acc": flat 6-bank region
        # time-shared by slots^T accumulation (P2), y^T accumulation (P3),
        # and the triple-buffered combine outputs (P4).
        ps_small = pool("ps_small", 2, "PSUM")
        ps_acc = pool("ps_acc", 1, "PSUM")
        ACC = Dc * 512
        assert ACC * 4 <= 6 * 2048

        phiT_s = singles.tile([128, Dc, SL], BF16)
        nc.sync.dma_start(phiT_s[:], phiT.rearrange("k p m -> p k m"))
        b1_s = singles.tile([128, Hc], F32)
        nc.sync.dma_start(b1_s[:], b1.rearrange("o p -> p o"))
        # w1/w2 (9.4 MB) are DMAed after batch 0's phase-1 instructions are
        # issued, so the first x tiles aren't queued behind them; the load
        # still completes well before phase 3 needs the weights.
        w1_s = singles.tile([128, Dc, H], BF16)
        w2_s = singles.tile([128, Hc, D], BF16)
        ident = singles.tile([128, 128], F32)
        make_identity(nc, ident[:])
        zbias = singles.tile([128, 1], F32)
        nc.vector.memset(zbias[:], 0.0)

        for b in range(B):
            # ---- phase 1: logits + exp -> E^T tiles, with exp-sums ----
            eT_t = []
            ddp = dd_pool.tile([128, SLc, NT], F32)
            for t in range(NT):
                et = eT_pool.tile([128, SLc, 512], BF16, name="et")
                eT_t.append(et)
                xt = xT_pool.tile([128, Dc, 512], BF16)
                nc.sync.dma_start(
                    xt[:], xT[b, :, :, ts(t, 512)].rearrange("k p n -> p k n")
                )
                for s in range(SLc):
                    ps = ps_small.tile([128, 512], F32, tag="pss", name="psL")
                    for d in range(Dc):
                        nc.tensor.matmul(
                            ps[:],
                            phiT_s[:, d, ts(s, 128)],
                            xt[:, d, :],
                            start=(d == 0),
                            stop=(d == Dc - 1),
                        )
                    nc.scalar.activation(
                        et[:, s, :],
                        ps[:],
                        AF.Exp,
                        bias=zbias[:],
                        accum_out=ddp[:, s, t : t + 1],
                    )

            if b == 0:
                w1_r = w1.rearrange("(k p) m -> p k m", p=128)
                w2_r = w2.rearrange("(k p) m -> p k m", p=128)
                for d in range(Dc):
                    nc.sync.dma_start(w1_s[:, d, :], w1_r[:, d, :])
                for h4 in range(0, Hc, 4):
                    nc.sync.dma_start(
                        w2_s[:, h4 : h4 + 4, :], w2_r[:, h4 : h4 + 4, :]
                    )

            def eT_blk(s, v):
                return eT_t[v // 4][:, s, ts(v % 4, 128)]

            # ---- dispatch denominators -> per-s scaled identity ----
            rdd = dd_pool.tile([128, SLc], F32)
            nc.vector.reduce_sum(rdd[:], ddp[:], axis=mybir.AxisListType.X)
            nc.vector.reciprocal(rdd[:], rdd[:])
            diag = diag_pool.tile([128, SLc, 128], BF16)
            for s in range(SLc):
                nc.vector.tensor_scalar_mul(
                    diag[:, s, :], ident[:], rdd[:, s : s + 1]
                )
            # ---- phase 2: dispatch transpose+normalize, slots^T matmul ----
            # The transpose-matmuls against diag(1/dd) fuse the softmax
            # normalization into the E^T block transposes. Software-pipelined
            # one v ahead so the PE runs v+1 transposes while the DVE drains
            # psDt(v) into Dt(v).
            accS = ps_acc.tile([128, ACC], F32, tag="acc", name="accS")

            def p2_transposes(v):
                psDt = ps_small.tile([128, 512], F32, tag="pss", name="psD")
                for s in range(SLc):
                    nc.tensor.matmul(
                        psDt[:, ts(s, 128)],
                        eT_blk(s, v),
                        diag[:, s, :],
                        start=True,
                        stop=True,
                    )
                Dt = D_pool.tile([128, SL], BF16)
                nc.vector.tensor_copy(Dt[:], psDt[:])
                xn = xN_pool.tile([128, D], BF16)
                nc.sync.dma_start(xn[:], xN[b, ts(v, 128), :])
                return Dt, xn

            def p2_slots(v, Dt, xn):
                for d in range(Dc):
                    nc.tensor.matmul(
                        accS[:, d * 512 : d * 512 + SL],
                        xn[:, ts(d, 128)],
                        Dt[:],
                        start=(v == 0),
                        stop=(v == NV - 1),
                    )

            pend = p2_transposes(0)
            for v in range(NV):
                nxt = p2_transposes(v + 1) if v + 1 < NV else None
                p2_slots(v, *pend)
                pend = nxt
            slotsT = slots_pool.tile([128, Dc, SL], BF16)
            for d in range(Dc):
                nc.vector.tensor_copy(
                    slotsT[:, d, :], accS[:, d * 512 : d * 512 + SL]
                )
            # ---- phase 3: expert MLP, y^T accumulation ----
            # y matmuls for h trail the h matmuls for h+1 by one step so the
            # PE never waits on gelu.
            accY = ps_acc.tile([128, ACC], F32, tag="acc", name="accY")

            def p3_h(h):
                psh = ps_small.tile([128, 512], F32, tag="pss", name="psH")
                for d in range(Dc):
                    nc.tensor.matmul(
                        psh[:, :SL],
                        w1_s[:, d, ts(h, 128)],
                        slotsT[:, d, :],
                        start=(d == 0),
                        stop=(d == Dc - 1),
                    )
                ht = h_pool.tile([128, SL], BF16)
                nc.scalar.activation(
                    ht[:], psh[:, :SL], act_fn, bias=b1_s[:, h : h + 1]
                )
                return ht

            def p3_y(h, ht):
                for d in range(Dc):
                    nc.tensor.matmul(
                        accY[:, d * 512 : d * 512 + SL],
                        w2_s[:, h, ts(d, 128)],
                        ht[:],
                        start=(h == 0),
                        stop=(h == Hc - 1),
                    )

            pend_h = p3_h(0)
            for h in range(Hc):
                nxt_h = p3_h(h + 1) if h + 1 < Hc else None
                p3_y(h, pend_h)
                pend_h = nxt_h
            # y^T -> y via one XBAR transpose: yTs free layout (s, d, i)
            # so yTT blocks come out (s, d)-ordered and slot block s reads
            # its 768 y columns contiguously. The 256-wide tail + the ones
            # column are staged into y_augB for the combine pB group.
            yTs = slots_pool.tile([128, SLc, Dc, 128], BF16, tag="yTs", name="yTs")
            for d in range(Dc):
                nc.vector.tensor_copy(
                    yTs[:, :, d, :],
                    accY[:, d * 512 : d * 512 + SL].rearrange(
                        "p (s k) -> p s k", s=SLc
                    ),
                )
            yTT = y_pool.tile([128, SLc * Dc, 128], BF16, tag="yTT", name="yTT")
            nc.sync.dma_start_transpose(yTT[:], yTs[:])
            y_augB = y_pool.tile([128, SLc, OD - 512], BF16, tag="yB", name="yB")
            nc.vector.memset(y_augB[:, :, 256 : 257], 1.0)
            nc.vector.memset(y_augB[:, :, 257 : 258], 0.0)
            nc.vector.tensor_copy(
                y_augB[:, :, 0:256],
                yTT[:, :, :].rearrange("p (s d) k -> p s (d k)", s=SLc)[
                    :, :, 512:D
                ],
            )
            # ---- phase 4: combine partials + local denominator ----
            # Triple-buffered by column ranges of the 6-bank acc region; the
            # drain copies go to separate tiles on separate engines.
            psC = ps_acc.tile([128, ACC], F32, tag="acc", name="psC")

            def p4_mms(v):
                base = (v % 3) * 1024
                pA = psC[:, base : base + 512]
                pB = psC[:, base + 512 : base + OD]
                for s in range(SLc):
                    nc.tensor.matmul(
                        pA,
                        eT_blk(s, v),
                        yTT[:, ts(s, Dc), :].rearrange("p d k -> p (d k)")[
                            :, 0:512
                        ],
                        start=(s == 0),
                        stop=(s == SLc - 1),
                    )
                    nc.tensor.matmul(
                        pB,
                        eT_blk(s, v),
                        y_augB[:, s, :],
                        start=(s == 0),
                        stop=(s == SLc - 1),
                    )
                return pA, pB

            def p4_drain(v, pA, pB):
                ot = out_pool.tile([128, OD], F32, tag="ot", name="ot")
                base = (v % 3) * 1024
                if v % 3 == 0:
                    nc.scalar.copy(ot[:], psC[:, base : base + OD])
                else:
                    nc.vector.tensor_copy(ot[:], psC[:, base : base + OD])
                nc.sync.dma_start(outp[b, ts(v, 128), :], ot[:])

            # Groups of three v-blocks fill the three 2-bank regions, then
            # drain on three different engines in parallel; only the slowest
            # drain gates the next group.
            for v0 in range(0, NV, 3):
                grp = [(v, p4_mms(v)) for v in range(v0, min(v0 + 3, NV))]
                for v, ps in grp:
                    p4_drain(v, *ps)

    return nc


def _to_bf16(a):
    import ml_dtypes

    return np.asarray(a, dtype=np.float32).astype(ml_dtypes.bfloat16)


def _make_core_inputs(x, phi, w1, b1, w2, n_cores=N_CORES):
    B, N, Dd = x.shape
    E, S, _ = phi.shape
    H = w1.shape[2]
    halves = n_cores // E
    SL = S // halves
    Dc, Hc = Dd // 128, H // 128
    xT_full = _to_bf16(
        np.ascontiguousarray(x.transpose(0, 2, 1)).reshape(B, Dc, 128, N)
    )
    x_c = _to_bf16(x)
    w1_b = [_to_bf16(w1[e]) for e in range(E)]
    w2_b = [_to_bf16(w2[e]) for e in range(E)]
    b1_b = [np.ascontiguousarray(b1[e]).reshape(Hc, 128) for e in range(E)]
    in_maps = []
    for c in range(n_cores):
        e, hh = c // halves, c % halves
        phi_loc = phi[e, hh * SL : (hh + 1) * SL, :]
        phiT = _to_bf16(np.ascontiguousarray(phi_loc.T).reshape(Dc, 128, SL))
        in_maps.append(
            {
                "xT": xT_full,
                "xN": x_c,
                "phiT": phiT,
                "w1": w1_b[e],
                "w2": w2_b[e],
                "b1": b1_b[e],
            }
        )
    return in_maps


def _combine_core_outputs(outs, b2, n_cores=N_CORES):
    E, D = b2.shape
    halves = n_cores // E
    num = np.zeros(outs[0]["outp"][..., :D].shape, dtype=np.float64)
    den = np.zeros(outs[0]["outp"][..., D].shape, dtype=np.float64)
    for c, r in enumerate(outs):
        e = c // halves
        gdl = r["outp"][..., D].astype(np.float64)
        num += r["outp"][..., :D]
        num += gdl[..., None] * b2[e].astype(np.float64)[None, None, :]
        den += gdl
    return (num / den[..., None]).astype(np.float32)


def build_nc(x, phi, w1, b1, w2, b2):
    B, N, D = x.shape
    E, S, _ = phi.shape
    H = w1.shape[2]
    SL = S // (N_CORES // E)

    nc = bass.Bass(
        "TRN2", target_bir_lowering=False, debug=False, num_devices=N_CORES
    )
    _emit_moe_kernel(nc, B, N, D, SL, H)
    _split_excess_waits(nc)
    return nc


def make_in_maps(x, phi, w1, b1, w2, b2):
    return _make_core_inputs(
        np.asarray(x, dtype=np.float32),
        np.asarray(phi, dtype=np.float32),
        np.asarray(w1, dtype=np.float32),
        np.asarray(b1, dtype=np.float32),
        np.asarray(w2, dtype=np.float32),
    )


def combine_outputs(results, x, phi, w1, b1, w2, b2):
    return _combine_core_outputs(results, np.asarray(b2, dtype=np.float32))


def kernel(x, phi, w1, b1, w2, b2):
    x = np.asarray(x, dtype=np.float32)
    phi = np.asarray(phi, dtype=np.float32)
    w1 = np.asarray(w1, dtype=np.float32)
    b1 = np.asarray(b1, dtype=np.float32)
    w2 = np.asarray(w2, dtype=np.float32)
    b2 = np.asarray(b2, dtype=np.float32)

    nc = build_nc(x, phi, w1, b1, w2, b2)
    in_maps = make_in_maps(x, phi, w1, b1, w2, b2)
    res = run_bass_kernel_spmd(nc, in_maps, core_ids=list(range(N_CORES)))
    return combine_outputs(res.results, x, phi, w1, b1, w2, b2)
